# revision 9
# baseline (speedup 1.0000x reference)
"""CRAM block Trainium2 kernel (Bass/Tile), 8-core SPMD — v2 fused.

Shard: core i -> (batch b=i//2, seq-half i%2): T=2048 tokens + 128-token halo.

v2 design (vs v1): all matmuls in bf16 (same PE cycle cost as f32r, exact
enough at rel-tol 2e-2), FFN fused per 512-token block with g^T kept in
SBUF (no 64 MiB DRAM round-trip), W1 resident bf16, W2 streamed per block,
phase A software-pipelined so PE never stalls on the sigmoid/LN chain
(keeps HAM at 2.4 GHz), PSUM->SBUF copies and LN-apply offloaded to the
Scalar engine, LN scale/bias to GpSimd.

Phases (per core):
  A (17 chunks of 128 tokens, SW-pipelined by 2):
     x -> PE-transpose -> xT(bf16) -> psig = xT.T@Wret -> +bret -> sigmoid
     -> sig(bf16); EMA r = L@sig_c + U@sig_{c-1} (triangular matmuls);
     LN1(x+r) = h(bf16) -> DRAM spill + PE-transpose into resident
     hT [128, KH, T] bf16.
  B/C per 512-token block:
     B: for f in 32: g[f] = gelu(W1[:,f].T @ hT_blk + b1[f]) bf16 in SBUF.
     C: two n-half passes: psum[t] = sum_f g[f][t].T @ W2[f, nhalf];
        v2 = psum + h + b2; LN2 -> out.

EMA-as-matmul: decay 0.5 -> contributions >256 steps back are below fp32
resolution; L^T[j,t]=0.5^(t-j+1) (j<=t), U^T[j,t]=0.5^(t+129-j). Cores at
seq start get U0=0 (no halo).
"""
import sys
sys.path.insert(0, '/opt/trn_rl_repo')

from contextlib import ExitStack

import numpy as np
import ml_dtypes
import concourse.bass as bass
import concourse.tile as tile
from concourse import mybir, bacc
import time
import jax
from jax.sharding import Mesh, PartitionSpec
from jax.experimental.shard_map import shard_map
from concourse.bass2jax import _bass_exec_p, partition_id_tensor, install_neuronx_cc_hook


F32 = mybir.dt.float32
BF16 = mybir.dt.bfloat16
AF = mybir.ActivationFunctionType
ALU = mybir.AluOpType

B, S, H, FF = 4, 4096, 1024, 4096
EPS = 1e-5
N_CORES = 8
T = 2048            # tokens per core
TC = T // 128       # 16 output chunks
TCI = TC + 1        # incl. halo chunk
KH = H // 128       # 8 h chunks
KF = FF // 128      # 32 f chunks
NT = T // 512       # 4 token blocks of 512
GELU = AF.Gelu_apprx_tanh   # jax.nn.gelu default is approximate=True (tanh)


def build_nc(repeat=1, debug_taps=False):
    nc = bacc.Bacc("TRN2", target_bir_lowering=False, debug=False,
                   num_devices=N_CORES)

    x_in = nc.dram_tensor("x", [TCI * 128, H], F32, kind="ExternalInput")
    wret_in = nc.dram_tensor("wret", [H, H], BF16, kind="ExternalInput")
    w1_in = nc.dram_tensor("w1", [H, FF], BF16, kind="ExternalInput")
    w2_in = nc.dram_tensor("w2", [FF, H], BF16, kind="ExternalInput")
    bret_in = nc.dram_tensor("bret", [128, H], F32, kind="ExternalInput")
    b2_in = nc.dram_tensor("b2", [128, H], F32, kind="ExternalInput")
    lns1_in = nc.dram_tensor("lns1", [128, H], F32, kind="ExternalInput")
    lnb1_in = nc.dram_tensor("lnb1", [128, H], F32, kind="ExternalInput")
    lns2_in = nc.dram_tensor("lns2", [128, H], F32, kind="ExternalInput")
    lnb2_in = nc.dram_tensor("lnb2", [128, H], F32, kind="ExternalInput")
    b1_in = nc.dram_tensor("b1", [128, KF], F32, kind="ExternalInput")
    ema_l_in = nc.dram_tensor("ema_l", [128, 128], BF16, kind="ExternalInput")
    ema_u_in = nc.dram_tensor("ema_u", [128, 128], BF16, kind="ExternalInput")
    ema_u0_in = nc.dram_tensor("ema_u0", [128, 128], BF16, kind="ExternalInput")
    ident_in = nc.dram_tensor("ident", [128, 128], F32, kind="ExternalInput")
    identb_in = nc.dram_tensor("identb", [128, 128], BF16, kind="ExternalInput")

    out_t = nc.dram_tensor("out", [T, H], F32, kind="ExternalOutput")

    ins = dict(x=x_in, wret=wret_in, w1=w1_in, w2=w2_in, bret=bret_in,
               b2=b2_in, lns1=lns1_in, lnb1=lnb1_in, lns2=lns2_in,
               lnb2=lnb2_in, b1=b1_in, ema_l=ema_l_in, ema_u=ema_u_in,
               ema_u0=ema_u0_in, ident=ident_in, identb=identb_in)
    if debug_taps:
        ins["h_dbg"] = nc.dram_tensor("h_dbg", [TC, 128, H], F32, kind="ExternalOutput")
        ins["sig_dbg"] = nc.dram_tensor("sig_dbg", [TCI, 128, H], BF16, kind="ExternalOutput")
        ins["g_dbg"] = nc.dram_tensor("g_dbg", [NT, KF, 128, 512], BF16, kind="ExternalOutput")

    with tile.TileContext(nc) as tc:
        with ExitStack() as octx:
            singles = octx.enter_context(tc.tile_pool(name="singles", bufs=1))
            cst = load_constants(tc, singles, ins)
            for _ in range(repeat):
                one_pass(tc, cst, ins, out_t, debug_taps=debug_taps)
    nc.compile()
    return nc


def load_constants(tc, singles, ins):
    nc = tc.nc
    cst = {}

    def load(name, shape, dt, src):
        t = singles.tile(shape, dt, name=name, tag=name)
        nc.sync.dma_start(out=t[:], in_=src)
        cst[name] = t
        return t

    load("ident", [128, 128], F32, ins["ident"][:])
    load("identb", [128, 128], BF16, ins["identb"][:])
    load("ema_l", [128, 128], BF16, ins["ema_l"][:])
    load("ema_u", [128, 128], BF16, ins["ema_u"][:])
    load("ema_u0", [128, 128], BF16, ins["ema_u0"][:])
    for nm in ["bret", "b2", "lns1", "lnb1", "lns2", "lnb2"]:
        load(nm, [128, H], F32, ins[nm][:])
    load("b1", [128, KF], F32, ins["b1"][:])
    eps_t = singles.tile([128, 1], F32)
    nc.vector.memset(eps_t[:], EPS)
    cst["eps"] = eps_t
    return cst


def layernorm(nc, spool, v, out, cst, scale_bc, bias_bc):
    """out = (v - mean)/sqrt(var+eps) * scale + bias over free dim (H).

    bn stats on DVE; normalize-apply on Scalar (per-partition scale/bias);
    gamma/beta apply on GpSimd. v is f32 SBUF [128, H]; out may be bf16.
    """
    stats = spool.tile([128, 2, 6], F32, tag="ln_stats")
    v2 = v[:].rearrange("p (s q) -> p s q", s=2)
    for s in range(2):
        nc.vector.bn_stats(out=stats[:, s, :], in_=v2[:, s, :])
    mv = spool.tile([128, 2], F32, tag="ln_mv")
    nc.vector.bn_aggr(out=mv[:], in_=stats[:])
    std = spool.tile([128, 1], F32, tag="ln_std")
    nc.scalar.activation(out=std[:], in_=mv[:, 1:2], func=AF.Sqrt,
                         bias=cst["eps"][:], scale=1.0)
    rstd = spool.tile([128, 1], F32, tag="ln_rstd")
    nc.vector.reciprocal(out=rstd[:], in_=std[:])
    nmr = spool.tile([128, 1], F32, tag="ln_nmr")
    nc.vector.scalar_tensor_tensor(out=nmr[:], in0=mv[:, 0:1], scalar=-1.0,
                                   in1=rstd[:], op0=ALU.mult, op1=ALU.mult)
    nc.scalar.activation(out=out[:], in_=v[:], func=AF.Identity,
                         bias=nmr[:], scale=rstd[:])
    nc.gpsimd.tensor_mul(out=out[:], in0=out[:], in1=scale_bc[:])
    nc.gpsimd.tensor_add(out=out[:], in0=out[:], in1=bias_bc[:])


def one_pass(tc, cst, ins, out_t, debug_taps=False):
    nc = tc.nc
    with ExitStack() as octx:
        dram = octx.enter_context(tc.tile_pool(name="dram", bufs=1, space="DRAM"))
        h_scr = dram.tile([TC, 128, H], F32)

        big = octx.enter_context(tc.tile_pool(name="big", bufs=1))
        hT = big.tile([128, KH, T], BF16, tag="hT")
        w1_sb = big.tile([128, KH, FF], BF16, tag="w1sb")

        # ---------------- Phase A (software-pipelined chunks) ----------------
        with ExitStack() as ctx:
            wr_pool = ctx.enter_context(tc.tile_pool(name="wretp", bufs=1))
            wret_sb = wr_pool.tile([128, KH, H], BF16)
            for k in range(KH):
                nc.sync.dma_start(out=wret_sb[:, k, :],
                                  in_=ins["wret"][k * 128:(k + 1) * 128, :])
            for k in range(KH):
                nc.sync.dma_start(out=w1_sb[:, k, :],
                                  in_=ins["w1"][k * 128:(k + 1) * 128, :])

            pa_x = ctx.enter_context(tc.tile_pool(name="pa_x", bufs=3))
            pa_xT = ctx.enter_context(tc.tile_pool(name="pa_xT", bufs=2))
            pa_sig = ctx.enter_context(tc.tile_pool(name="pa_sig", bufs=4))
            pa_h = ctx.enter_context(tc.tile_pool(name="pa_h", bufs=3))
            pa_ln = ctx.enter_context(tc.tile_pool(name="pa_ln", bufs=3))
            ps_t = ctx.enter_context(tc.tile_pool(name="ps_t", bufs=2, space="PSUM"))
            ps_sig = ctx.enter_context(tc.tile_pool(name="ps_sig", bufs=2, space="PSUM"))
            ps_r = ctx.enter_context(tc.tile_pool(name="ps_r", bufs=1, space="PSUM"))

            xcs, sigs, hs = {}, {}, {}
            # iter c: sig path for chunk c; EMA+LN for chunk c-1; hT transp c-2
            for c in range(TCI + 2):
                if c < TCI:
                    xc = pa_x.tile([128, H], F32, tag="xc")
                    nc.sync.dma_start(out=xc[:], in_=ins["x"][c * 128:(c + 1) * 128, :])
                    xcs[c] = xc
                    xT = pa_xT.tile([128, KH, 128], BF16, tag="xT")
                    for k2 in range(2):
                        pt = ps_t.tile([128, 4, 128], F32, tag="pt")
                        for j in range(4):
                            k = k2 * 4 + j
                            nc.tensor.transpose(pt[:, j, :],
                                                xc[:, k * 128:(k + 1) * 128],
                                                cst["ident"][:])
                        for j in range(4):
                            k = k2 * 4 + j
                            nc.scalar.copy(out=xT[:, k, :], in_=pt[:, j, :])
                    psig = ps_sig.tile([128, H], F32, tag="psig")
                    for k in range(KH):
                        for n in range(2):
                            nc.tensor.matmul(
                                psig[:, n * 512:(n + 1) * 512],
                                xT[:, k, :],
                                wret_sb[:, k, n * 512:(n + 1) * 512],
                                start=(k == 0), stop=(k == KH - 1),
                                skip_group_check=True,
                            )
                    nc.vector.tensor_add(out=psig[:], in0=psig[:], in1=cst["bret"][:])
                    sig = pa_sig.tile([128, H], BF16, tag="sig")
                    nc.scalar.activation(out=sig[:], in_=psig[:], func=AF.Sigmoid)
                    sigs[c] = sig
                    if debug_taps:
                        nc.sync.dma_start(out=ins["sig_dbg"][c], in_=sig[:])

                j = c - 1
                if 1 <= j < TCI:
                    # EMA for chunk j (output index j-1)
                    pr = ps_r.tile([128, H], F32, tag="pr")
                    uu = cst["ema_u0"] if j == 1 else cst["ema_u"]
                    for n in range(2):
                        sl = slice(n * 512, (n + 1) * 512)
                        nc.tensor.matmul(pr[:, sl], cst["ema_l"][:], sigs[j][:, sl],
                                         start=True, stop=False, skip_group_check=True)
                    for n in range(2):
                        sl = slice(n * 512, (n + 1) * 512)
                        nc.tensor.matmul(pr[:, sl], uu[:], sigs[j - 1][:, sl],
                                         start=False, stop=True, skip_group_check=True)
                    # v = r + x (in-place into xc[j])
                    nc.vector.tensor_add(out=xcs[j][:], in0=pr[:], in1=xcs[j][:])
                    hc = pa_h.tile([128, H], F32, tag="hc")
                    layernorm(nc, pa_ln, xcs[j], hc, cst, cst["lns1"], cst["lnb1"])
                    nc.sync.dma_start(out=h_scr[j - 1], in_=hc[:])
                    if debug_taps:
                        nc.sync.dma_start(out=ins["h_dbg"][j - 1], in_=hc[:])
                    hs[j] = hc
                    sigs.pop(j - 1, None)

                j2 = c - 2
                if 1 <= j2 < TCI:
                    # hT transposes for chunk j2 (output index j2-1)
                    hc = hs.pop(j2)
                    for k2 in range(2):
                        pt = ps_t.tile([128, 4, 128], F32, tag="pt")
                        for j in range(4):
                            k = k2 * 4 + j
                            nc.tensor.transpose(pt[:, j, :],
                                                hc[:, k * 128:(k + 1) * 128],
                                                cst["ident"][:])
                        for j in range(4):
                            k = k2 * 4 + j
                            nc.vector.tensor_copy(
                                out=hT[:, k, (j2 - 1) * 128:j2 * 128],
                                in_=pt[:, j, :])

        # ---------------- Phase B/C fused per 512-token block ----------------
        with ExitStack() as ctx:
            pb_g = ctx.enter_context(tc.tile_pool(name="pb_g", bufs=34))
            pc_w2 = ctx.enter_context(tc.tile_pool(name="pc_w2", bufs=17))
            pc_h = ctx.enter_context(tc.tile_pool(name="pc_h", bufs=4))
            pc_v = ctx.enter_context(tc.tile_pool(name="pc_v", bufs=4))
            pc_ln = ctx.enter_context(tc.tile_pool(name="pc_ln", bufs=3))
            ps_g = ctx.enter_context(tc.tile_pool(name="ps_g", bufs=2, space="PSUM"))
            ps_o = ctx.enter_context(tc.tile_pool(name="ps_o", bufs=4, space="PSUM"))

            for blk in range(NT):
                toff = blk * 512
                # --- B: g[f] = gelu(W1[:,f].T @ hT_blk + b1[f]), bf16 ---
                gts = []
                for f in range(KF):
                    pg = ps_g.tile([128, 512], F32, tag="pg")
                    for k in range(KH):
                        nc.tensor.matmul(
                            pg[:],
                            w1_sb[:, k, f * 128:(f + 1) * 128],
                            hT[:, k, toff:toff + 512],
                            start=(k == 0), stop=(k == KH - 1),
                            skip_group_check=True,
                        )
                    g = pb_g.tile([128, 512], BF16, tag="g")
                    nc.scalar.activation(out=g[:], in_=pg[:], func=GELU,
                                         bias=cst["b1"][:, f:f + 1], scale=1.0)
                    if debug_taps:
                        nc.sync.dma_start(out=ins["g_dbg"][blk, f], in_=g[:])
                    gts.append(g)

                # --- C: out = LN2(sum_f g^T W2 + h + b2) ---
                hts, v2s = [], []
                for t in range(4):
                    ht = pc_h.tile([128, H], F32, tag="ht")
                    nc.sync.dma_start(out=ht[:], in_=h_scr[blk * 4 + t])
                    hts.append(ht)
                    v2 = pc_v.tile([128, H], F32, tag="v2")
                    v2s.append(v2)
                for n in range(2):
                    sl = slice(n * 512, (n + 1) * 512)
                    pos = []
                    for _ in range(4):
                        po = ps_o.tile([128, 512], F32, tag="po", name="po")
                        pos.append(po)
                    for f in range(KF):
                        w2t = pc_w2.tile([128, 512], BF16, tag="w2t")
                        nc.sync.dma_start(
                            out=w2t[:],
                            in_=ins["w2"][f * 128:(f + 1) * 128, sl])
                        for t in range(4):
                            nc.tensor.matmul(
                                pos[t][:],
                                gts[f][:, t * 128:(t + 1) * 128],
                                w2t[:],
                                start=(f == 0), stop=(f == KF - 1),
                                skip_group_check=True,
                            )
                    for t in range(4):
                        # v2 = psum + h + b2
                        nc.vector.scalar_tensor_tensor(
                            out=v2s[t][:, sl], in0=pos[t][:], scalar=1.0,
                            in1=hts[t][:, sl], op0=ALU.mult, op1=ALU.add)
                        nc.vector.tensor_add(out=v2s[t][:, sl],
                                             in0=v2s[t][:, sl],
                                             in1=cst["b2"][:, sl])
                for t in range(4):
                    o = v2s[t]  # LN2 applied in place, then DMA out
                    layernorm(nc, pc_ln, v2s[t], o, cst, cst["lns2"], cst["lnb2"])
                    tt = blk * 4 + t
                    nc.sync.dma_start(out=out_t[tt * 128:(tt + 1) * 128, :], in_=o[:])


# ---------------------------------------------------------------------------
# Host side
# ---------------------------------------------------------------------------

def make_ema_mats():
    t = np.arange(128)
    j = np.arange(128)[:, None]
    Lt = np.where(j <= t[None, :], 0.5 ** (t[None, :] - j + 1.0), 0.0)
    Ut = 0.5 ** (t[None, :] + 129.0 - j)
    return Lt.astype(ml_dtypes.bfloat16), Ut.astype(ml_dtypes.bfloat16)


def make_in_maps(x, W_ret, b_ret, ln1_scale, ln1_bias, W1, b1, W2, b2,
                 ln2_scale, ln2_bias):
    Lt, Ut = make_ema_mats()
    bf = ml_dtypes.bfloat16
    bc = lambda vec: np.ascontiguousarray(
        np.broadcast_to(np.asarray(vec, np.float32)[None, :], (128, len(vec))))
    common = {
        "wret": np.ascontiguousarray(np.asarray(W_ret, np.float32).astype(bf)),
        "w1": np.ascontiguousarray(np.asarray(W1, np.float32).astype(bf)),
        "w2": np.ascontiguousarray(np.asarray(W2, np.float32).astype(bf)),
        "bret": bc(b_ret), "b2": bc(b2),
        "lns1": bc(ln1_scale), "lnb1": bc(ln1_bias),
        "lns2": bc(ln2_scale), "lnb2": bc(ln2_bias),
        "b1": np.ascontiguousarray(np.asarray(b1, np.float32).reshape(KF, 128).T),
        "ema_l": Lt,
        "ident": np.eye(128, dtype=np.float32),
        "identb": np.eye(128, dtype=np.float32).astype(bf),
    }
    in_maps = []
    for core in range(N_CORES):
        b, half = divmod(core, 2)
        xs = np.empty((TCI * 128, H), np.float32)
        if half == 0:
            xs[:128] = 0.0
            xs[128:] = x[b, 0:T]
            U = np.zeros_like(Ut)
        else:
            xs[:] = x[b, T - 128:S]
            U = Ut
        m = dict(common)
        m["x"] = xs
        m["ema_u"] = Ut
        m["ema_u0"] = U
        in_maps.append(m)
    return in_maps


def gather_out(results):
    out = np.empty((B, S, H), np.float32)
    for core in range(N_CORES):
        b, half = divmod(core, 2)
        out[b, half * T:(half + 1) * T] = results[core]["out"]
    return out


class SpmdRunner:
    def __init__(self, nc, n_cores):
        install_neuronx_cc_hook()
        self.nc = nc
        self.n_cores = n_cores
        assert nc.dbg_addr is None or not nc.dbg_callbacks

        in_names, out_names, out_avals, zero_outs = [], [], [], []
        partition_name = nc.partition_id_tensor.name if nc.partition_id_tensor else None
        for alloc in nc.m.functions[0].allocations:
            if not isinstance(alloc, mybir.MemoryLocationSet):
                continue
            name = alloc.memorylocations[0].name
            if alloc.kind == "ExternalInput":
                if name != partition_name:
                    in_names.append(name)
            elif alloc.kind == "ExternalOutput":
                shape = tuple(alloc.tensor_shape)
                dtype = mybir.dt.np(alloc.dtype)
                out_names.append(name)
                out_avals.append(jax.core.ShapedArray(shape, dtype))
                zero_outs.append(np.zeros(shape, dtype))
        if nc.dbg_addr is not None:
            self.dbg_name = nc.dbg_addr.name
        else:
            self.dbg_name = None
        self.in_names = list(in_names)
        self.out_names = out_names
        self.out_avals = out_avals
        self.zero_outs = zero_outs
        self.partition_name = partition_name
        n_params = len(self.in_names)
        n_outs = len(out_names)

        all_in_names = list(self.in_names) + list(out_names)
        if partition_name is not None:
            all_in_names.append(partition_name)

        def _body(*args):
            operands = list(args)
            if partition_name is not None:
                operands.append(partition_id_tensor())
            outs = _bass_exec_p.bind(
                *operands,
                out_avals=tuple(out_avals),
                in_names=tuple(all_in_names),
                out_names=tuple(out_names),
                lowering_input_output_aliases=(),
                sim_require_finite=True,
                sim_require_nnan=True,
                nc=nc,
            )
            return tuple(outs)

        devices = jax.devices()[:n_cores]
        assert len(devices) == n_cores
        self.mesh = Mesh(np.asarray(devices), ("core",))
        in_specs = (PartitionSpec("core"),) * (n_params + n_outs)
        out_specs = (PartitionSpec("core"),) * n_outs
        self.fn = jax.jit(
            shard_map(_body, mesh=self.mesh, in_specs=in_specs,
                      out_specs=out_specs, check_rep=False),
            keep_unused=True,
        )
        self._dev_zeros = None

    def _concat(self, in_maps):
        per_core = [[np.asarray(m[name]) for name in self.in_names] for m in in_maps]
        return [np.concatenate([per_core[c][i] for c in range(self.n_cores)], axis=0)
                for i in range(len(self.in_names))]

    def put(self, in_maps):
        concat_in = self._concat(in_maps)
        dev_in = [jax.device_put(x) for x in concat_in]
        if self._dev_zeros is None:
            self._dev_zeros = [
                jax.device_put(np.zeros((self.n_cores * z.shape[0], *z.shape[1:]), z.dtype))
                for z in self.zero_outs
            ]
        return dev_in

    def run(self, dev_in):
        out = self.fn(*dev_in, *self._dev_zeros)
        jax.block_until_ready(out)
        return out

    def results(self, out_arrs):
        res = []
        for c in range(self.n_cores):
            res.append({
                name: np.asarray(out_arrs[i]).reshape(self.n_cores, *self.out_avals[i].shape)[c]
                for i, name in enumerate(self.out_names)
            })
        return res

    def time_exec(self, dev_in, n=5):
        ts = []
        for _ in range(n):
            t0 = time.perf_counter()
            self.run(dev_in)
            ts.append(time.perf_counter() - t0)
        return min(ts), ts


# ---------------------------------------------------------------------------
# Public entry point: full inputs in, full output out.
# ---------------------------------------------------------------------------

_CACHE = {}


def kernel(x, W_ret, b_ret, ln1_scale, ln1_bias, W1, b1, W2, b2,
           ln2_scale, ln2_bias):
    """CRAM block on 8 Trainium2 NeuronCores. Full [4,4096,1024] in/out."""
    if "runner" not in _CACHE:
        nc = build_nc(repeat=1)
        _CACHE["runner"] = SpmdRunner(nc, N_CORES)
    runner = _CACHE["runner"]
    in_maps = make_in_maps(x, W_ret, b_ret, ln1_scale, ln1_bias, W1, b1,
                           W2, b2, ln2_scale, ln2_bias)
    dev_in = runner.put(in_maps)
    results = runner.results(runner.run(dev_in))
    return gather_out(results).astype(np.float32)


# revision 18
# speedup vs baseline: 157.6491x; 157.6491x over previous
"""CRAM block Trainium2 kernel (Bass/Tile), 8-core SPMD — v2 fused.

Shard: core i -> (batch b=i//2, seq-half i%2): T=2048 tokens + 128-token halo.

v2 design (vs v1): all matmuls in bf16 (same PE cycle cost as f32r, exact
enough at rel-tol 2e-2), FFN fused per 512-token block with g^T kept in
SBUF (no 64 MiB DRAM round-trip), W1 resident bf16, W2 streamed per block,
phase A software-pipelined so PE never stalls on the sigmoid/LN chain
(keeps HAM at 2.4 GHz), PSUM->SBUF copies and LN-apply offloaded to the
Scalar engine, LN scale/bias to GpSimd.

Phases (per core):
  A (17 chunks of 128 tokens, SW-pipelined by 2):
     x -> PE-transpose -> xT(bf16) -> psig = xT.T@Wret -> +bret -> sigmoid
     -> sig(bf16); EMA r = L@sig_c + U@sig_{c-1} (triangular matmuls);
     LN1(x+r) = h(bf16) -> DRAM spill + PE-transpose into resident
     hT [128, KH, T] bf16.
  B/C per 512-token block:
     B: for f in 32: g[f] = gelu(W1[:,f].T @ hT_blk + b1[f]) bf16 in SBUF.
     C: two n-half passes: psum[t] = sum_f g[f][t].T @ W2[f, nhalf];
        v2 = psum + h + b2; LN2 -> out.

EMA-as-matmul: decay 0.5 -> contributions >256 steps back are below fp32
resolution; L^T[j,t]=0.5^(t-j+1) (j<=t), U^T[j,t]=0.5^(t+129-j). Cores at
seq start get U0=0 (no halo).
"""
import sys
sys.path.insert(0, '/opt/trn_rl_repo')

from contextlib import ExitStack

import numpy as np
import ml_dtypes
import concourse.bass as bass
import concourse.tile as tile
from concourse import mybir, bacc
import time
import jax
from jax.sharding import Mesh, PartitionSpec
from jax.experimental.shard_map import shard_map
from concourse.bass2jax import _bass_exec_p, partition_id_tensor, install_neuronx_cc_hook


F32 = mybir.dt.float32
BF16 = mybir.dt.bfloat16
AF = mybir.ActivationFunctionType
ALU = mybir.AluOpType

B, S, H, FF = 4, 4096, 1024, 4096
EPS = 1e-5
N_CORES = 8
T = 2048            # tokens per core
TC = T // 128       # 16 output chunks
TCI = TC + 1        # incl. halo chunk
KH = H // 128       # 8 h chunks
KF = FF // 128      # 32 f chunks
NT = T // 512       # 4 token blocks of 512
GELU = AF.Gelu_apprx_tanh   # jax.nn.gelu default is approximate=True (tanh)


def build_nc(repeat=1, debug_taps=False):
    nc = bacc.Bacc("TRN2", target_bir_lowering=False, debug=False,
                   num_devices=N_CORES)

    x_in = nc.dram_tensor("x", [TCI * 128, H], F32, kind="ExternalInput")
    wret_in = nc.dram_tensor("wret", [H, H], BF16, kind="ExternalInput")
    w1_in = nc.dram_tensor("w1", [H, FF], BF16, kind="ExternalInput")
    w2_in = nc.dram_tensor("w2", [FF, H], BF16, kind="ExternalInput")
    bret_in = nc.dram_tensor("bret", [128, H], F32, kind="ExternalInput")
    b2_in = nc.dram_tensor("b2", [128, H], F32, kind="ExternalInput")
    lns1_in = nc.dram_tensor("lns1", [128, H], F32, kind="ExternalInput")
    lnb1_in = nc.dram_tensor("lnb1", [128, H], F32, kind="ExternalInput")
    lns2_in = nc.dram_tensor("lns2", [128, H], F32, kind="ExternalInput")
    lnb2_in = nc.dram_tensor("lnb2", [128, H], F32, kind="ExternalInput")
    b1_in = nc.dram_tensor("b1", [128, KF], F32, kind="ExternalInput")
    ema_l_in = nc.dram_tensor("ema_l", [128, 128], BF16, kind="ExternalInput")
    ema_u_in = nc.dram_tensor("ema_u", [128, 128], BF16, kind="ExternalInput")
    ema_u0_in = nc.dram_tensor("ema_u0", [128, 128], BF16, kind="ExternalInput")
    ident_in = nc.dram_tensor("ident", [128, 128], F32, kind="ExternalInput")
    identb_in = nc.dram_tensor("identb", [128, 128], BF16, kind="ExternalInput")

    out_t = nc.dram_tensor("out", [T, H], F32, kind="ExternalOutput")

    ins = dict(x=x_in, wret=wret_in, w1=w1_in, w2=w2_in, bret=bret_in,
               b2=b2_in, lns1=lns1_in, lnb1=lnb1_in, lns2=lns2_in,
               lnb2=lnb2_in, b1=b1_in, ema_l=ema_l_in, ema_u=ema_u_in,
               ema_u0=ema_u0_in, ident=ident_in, identb=identb_in)
    if debug_taps:
        ins["h_dbg"] = nc.dram_tensor("h_dbg", [TC, 128, H], F32, kind="ExternalOutput")
        ins["sig_dbg"] = nc.dram_tensor("sig_dbg", [TCI, 128, H], BF16, kind="ExternalOutput")
        ins["g_dbg"] = nc.dram_tensor("g_dbg", [NT, KF, 128, 512], BF16, kind="ExternalOutput")

    with tile.TileContext(nc) as tc:
        with ExitStack() as octx:
            singles = octx.enter_context(tc.tile_pool(name="singles", bufs=1))
            cst = load_constants(tc, singles, ins)
            for _ in range(repeat):
                one_pass(tc, cst, ins, out_t, debug_taps=debug_taps)
    nc.compile()
    return nc


def load_constants(tc, singles, ins):
    nc = tc.nc
    cst = {}

    def load(name, shape, dt, src):
        t = singles.tile(shape, dt, name=name, tag=name)
        nc.sync.dma_start(out=t[:], in_=src)
        cst[name] = t
        return t

    load("ident", [128, 128], F32, ins["ident"][:])
    load("identb", [128, 128], BF16, ins["identb"][:])
    load("ema_l", [128, 128], BF16, ins["ema_l"][:])
    load("ema_u", [128, 128], BF16, ins["ema_u"][:])
    load("ema_u0", [128, 128], BF16, ins["ema_u0"][:])
    for nm in ["bret", "b2", "lns1", "lnb1", "lns2", "lnb2"]:
        load(nm, [128, H], F32, ins[nm][:])
    load("b1", [128, KF], F32, ins["b1"][:])
    eps_t = singles.tile([128, 1], F32)
    nc.vector.memset(eps_t[:], EPS)
    cst["eps"] = eps_t
    return cst


def layernorm(nc, spool, v, out, cst, scale_bc, bias_bc, beta_gp=False):
    """out = (v - mean)/sqrt(var+eps) * scale + bias over free dim (H).

    All on DVE (no Scalar-engine table thrash): rstd = (var+eps)^-0.5 via
    tensor_scalar pow; apply = (v-mu)*rstd via tensor_scalar with two AP
    scalars. gamma on DVE; beta on GpSimd when beta_gp (phase A balance).
    """
    stats = spool.tile([128, 2, 6], F32, tag="ln_stats")
    v2 = v[:].rearrange("p (s q) -> p s q", s=2)
    for s in range(2):
        nc.vector.bn_stats(out=stats[:, s, :], in_=v2[:, s, :])
    mv = spool.tile([128, 2], F32, tag="ln_mv")
    nc.vector.bn_aggr(out=mv[:], in_=stats[:])
    std = spool.tile([128, 1], F32, tag="ln_std")
    nc.scalar.activation(out=std[:], in_=mv[:, 1:2], func=AF.Sqrt,
                         bias=cst["eps"][:], scale=1.0)
    rstd = spool.tile([128, 1], F32, tag="ln_rstd")
    nc.vector.reciprocal(out=rstd[:], in_=std[:])
    nc.vector.tensor_scalar(out=out[:], in0=v[:], scalar1=mv[:, 0:1],
                            scalar2=rstd[:],
                            op0=ALU.subtract, op1=ALU.mult)
    if beta_gp:
        nc.gpsimd.tensor_mul(out=out[:], in0=out[:], in1=scale_bc[:])
        nc.gpsimd.tensor_add(out=out[:], in0=out[:], in1=bias_bc[:])
    else:
        nc.vector.tensor_mul(out=out[:], in0=out[:], in1=scale_bc[:])
        nc.vector.tensor_add(out=out[:], in0=out[:], in1=bias_bc[:])


def one_pass(tc, cst, ins, out_t, debug_taps=False):
    nc = tc.nc
    with ExitStack() as octx:
        dram = octx.enter_context(tc.tile_pool(name="dram", bufs=1, space="DRAM"))
        h_scr = dram.tile([TC, 128, H], F32)

        big = octx.enter_context(tc.tile_pool(name="big", bufs=1))
        hT = big.tile([128, KH, T], BF16, tag="hT")
        w1_sb = big.tile([128, KH, FF], BF16, tag="w1sb")

        # ---------------- Phase A (software-pipelined chunks) ----------------
        with ExitStack() as ctx:
            pa_x = ctx.enter_context(tc.tile_pool(name="pa_x", bufs=4))
            pa_xT = ctx.enter_context(tc.tile_pool(name="pa_xT", bufs=2))
            pa_sig = ctx.enter_context(tc.tile_pool(name="pa_sig", bufs=4))
            pa_h = ctx.enter_context(tc.tile_pool(name="pa_h", bufs=3))
            pa_ln = ctx.enter_context(tc.tile_pool(name="pa_ln", bufs=3))
            ps_tx = ctx.enter_context(tc.tile_pool(name="ps_tx", bufs=2, space="PSUM"))
            ps_th = ctx.enter_context(tc.tile_pool(name="ps_th", bufs=2, space="PSUM"))
            ps_sig = ctx.enter_context(tc.tile_pool(name="ps_sig", bufs=2, space="PSUM"))
            ps_r = ctx.enter_context(tc.tile_pool(name="ps_r", bufs=1, space="PSUM"))

            xcs, sigs, hs = {}, {}, {}
            # Prefetch first x chunks BEFORE the weight loads so the PE can
            # start transposing immediately (weights otherwise head-block the
            # DMA queues for ~45us).
            PREF = 3
            for c in range(PREF):
                xc = pa_x.tile([128, H], F32, tag="xc", name="xc")
                nc.sync.dma_start(out=xc[:], in_=ins["x"][c * 128:(c + 1) * 128, :])
                xcs[c] = xc
            wr_pool = ctx.enter_context(tc.tile_pool(name="wretp", bufs=1))
            wret_sb = wr_pool.tile([128, KH, H], BF16)
            for k in range(KH):
                nc.sync.dma_start(out=wret_sb[:, k, :],
                                  in_=ins["wret"][k * 128:(k + 1) * 128, :])

            # iter c: sig path for chunk c; EMA+LN for chunk c-1; hT transp c-2
            for c in range(TCI + 2):
                if 1 <= c <= KH:
                    # spread the W1 load (needed only by phase B) across A
                    k = c - 1
                    nc.sync.dma_start(out=w1_sb[:, k, :],
                                      in_=ins["w1"][k * 128:(k + 1) * 128, :])
                if c < TCI:
                    if c >= PREF:
                        xc = pa_x.tile([128, H], F32, tag="xc", name="xc")
                        nc.sync.dma_start(out=xc[:], in_=ins["x"][c * 128:(c + 1) * 128, :])
                        xcs[c] = xc
                    xc = xcs[c]
                    xT = pa_xT.tile([128, KH, 128], BF16, tag="xT")
                    for k2 in range(2):
                        pt = ps_tx.tile([128, 4, 128], F32, tag="ptx")
                        for j in range(4):
                            k = k2 * 4 + j
                            nc.tensor.transpose(pt[:, j, :],
                                                xc[:, k * 128:(k + 1) * 128],
                                                cst["ident"][:])
                        for j in range(4):
                            k = k2 * 4 + j
                            nc.scalar.copy(out=xT[:, k, :], in_=pt[:, j, :])
                    sig = pa_sig.tile([128, H], BF16, tag="sig")
                    for n in range(2):
                        sl = slice(n * 512, (n + 1) * 512)
                        psig = ps_sig.tile([128, 512], F32, tag="psig")
                        for k in range(KH):
                            nc.tensor.matmul(
                                psig[:],
                                xT[:, k, :],
                                wret_sb[:, k, sl],
                                start=(k == 0), stop=(k == KH - 1),
                                skip_group_check=True,
                            )
                        nc.vector.tensor_add(out=psig[:], in0=psig[:],
                                             in1=cst["bret"][:, sl])
                        nc.scalar.activation(out=sig[:, sl], in_=psig[:],
                                             func=AF.Sigmoid)
                    sigs[c] = sig
                    if debug_taps:
                        nc.sync.dma_start(out=ins["sig_dbg"][c], in_=sig[:])

                j = c - 1
                if 1 <= j < TCI:
                    # EMA for chunk j (output index j-1)
                    pr = ps_r.tile([128, H], F32, tag="pr")
                    uu = cst["ema_u0"] if j == 1 else cst["ema_u"]
                    for n in range(2):
                        sl = slice(n * 512, (n + 1) * 512)
                        nc.tensor.matmul(pr[:, sl], cst["ema_l"][:], sigs[j][:, sl],
                                         start=True, stop=False, skip_group_check=True)
                    for n in range(2):
                        sl = slice(n * 512, (n + 1) * 512)
                        nc.tensor.matmul(pr[:, sl], uu[:], sigs[j - 1][:, sl],
                                         start=False, stop=True, skip_group_check=True)
                    # v = r + x (in-place into xc[j])
                    nc.vector.tensor_add(out=xcs[j][:], in0=pr[:], in1=xcs[j][:])
                    hc = pa_h.tile([128, H], F32, tag="hc")
                    layernorm(nc, pa_ln, xcs[j], hc, cst, cst["lns1"], cst["lnb1"], beta_gp=True)
                    nc.sync.dma_start(out=h_scr[j - 1], in_=hc[:])
                    if debug_taps:
                        nc.sync.dma_start(out=ins["h_dbg"][j - 1], in_=hc[:])
                    hs[j] = hc
                    sigs.pop(j - 1, None)

                j2 = c - 2
                if 1 <= j2 < TCI:
                    # hT transposes for chunk j2 (output index j2-1)
                    hc = hs.pop(j2)
                    for k2 in range(2):
                        pt = ps_th.tile([128, 4, 128], F32, tag="pth")
                        for j in range(4):
                            k = k2 * 4 + j
                            nc.tensor.transpose(pt[:, j, :],
                                                hc[:, k * 128:(k + 1) * 128],
                                                cst["ident"][:])
                        for j in range(4):
                            k = k2 * 4 + j
                            nc.vector.tensor_copy(
                                out=hT[:, k, (j2 - 1) * 128:j2 * 128],
                                in_=pt[:, j, :])

        # ---------------- Phase B/C fused per 512-token block ----------------
        with ExitStack() as ctx:
            pb_g = ctx.enter_context(tc.tile_pool(name="pb_g", bufs=34))
            pc_w2 = ctx.enter_context(tc.tile_pool(name="pc_w2", bufs=17))
            pc_h = ctx.enter_context(tc.tile_pool(name="pc_h", bufs=4))
            pc_v = ctx.enter_context(tc.tile_pool(name="pc_v", bufs=4))
            pc_ln = ctx.enter_context(tc.tile_pool(name="pc_ln", bufs=3))
            ps_g = ctx.enter_context(tc.tile_pool(name="ps_g", bufs=2, space="PSUM"))
            ps_o = ctx.enter_context(tc.tile_pool(name="ps_o", bufs=4, space="PSUM"))

            for blk in range(NT):
                toff = blk * 512
                # --- B: g[f] = gelu(W1[:,f].T @ hT_blk + b1[f]), bf16 ---
                gts = []
                for f in range(KF):
                    pg = ps_g.tile([128, 512], F32, tag="pg")
                    for k in range(KH):
                        nc.tensor.matmul(
                            pg[:],
                            w1_sb[:, k, f * 128:(f + 1) * 128],
                            hT[:, k, toff:toff + 512],
                            start=(k == 0), stop=(k == KH - 1),
                            skip_group_check=True,
                        )
                    g = pb_g.tile([128, 512], BF16, tag="g")
                    nc.scalar.activation(out=g[:], in_=pg[:], func=GELU,
                                         bias=cst["b1"][:, f:f + 1], scale=1.0)
                    if debug_taps:
                        nc.sync.dma_start(out=ins["g_dbg"][blk, f], in_=g[:])
                    gts.append(g)

                # --- C: out = LN2(sum_f g^T W2 + h + b2) ---
                hts, v2s = [], []
                for t in range(4):
                    ht = pc_h.tile([128, H], F32, tag="ht")
                    nc.sync.dma_start(out=ht[:], in_=h_scr[blk * 4 + t])
                    hts.append(ht)
                    v2 = pc_v.tile([128, H], F32, tag="v2")
                    v2s.append(v2)
                for n in range(2):
                    sl = slice(n * 512, (n + 1) * 512)
                    pos = []
                    for _ in range(4):
                        po = ps_o.tile([128, 512], F32, tag="po", name="po")
                        pos.append(po)
                    for f in range(KF):
                        w2t = pc_w2.tile([128, 512], BF16, tag="w2t")
                        nc.sync.dma_start(
                            out=w2t[:],
                            in_=ins["w2"][f * 128:(f + 1) * 128, sl])
                        for t in range(4):
                            nc.tensor.matmul(
                                pos[t][:],
                                gts[f][:, t * 128:(t + 1) * 128],
                                w2t[:],
                                start=(f == 0), stop=(f == KF - 1),
                                skip_group_check=True,
                            )
                    for t in range(4):
                        # v2 = psum + h + b2
                        nc.vector.scalar_tensor_tensor(
                            out=v2s[t][:, sl], in0=pos[t][:], scalar=1.0,
                            in1=hts[t][:, sl], op0=ALU.mult, op1=ALU.add)
                        nc.vector.tensor_add(out=v2s[t][:, sl],
                                             in0=v2s[t][:, sl],
                                             in1=cst["b2"][:, sl])
                for t in range(4):
                    o = v2s[t]  # LN2 applied in place, then DMA out
                    layernorm(nc, pc_ln, v2s[t], o, cst, cst["lns2"], cst["lnb2"])
                    tt = blk * 4 + t
                    nc.sync.dma_start(out=out_t[tt * 128:(tt + 1) * 128, :], in_=o[:])


# ---------------------------------------------------------------------------
# Host side
# ---------------------------------------------------------------------------

def make_ema_mats():
    t = np.arange(128)
    j = np.arange(128)[:, None]
    Lt = np.where(j <= t[None, :], 0.5 ** (t[None, :] - j + 1.0), 0.0)
    Ut = 0.5 ** (t[None, :] + 129.0 - j)
    return Lt.astype(ml_dtypes.bfloat16), Ut.astype(ml_dtypes.bfloat16)


def make_in_maps(x, W_ret, b_ret, ln1_scale, ln1_bias, W1, b1, W2, b2,
                 ln2_scale, ln2_bias):
    Lt, Ut = make_ema_mats()
    bf = ml_dtypes.bfloat16
    bc = lambda vec: np.ascontiguousarray(
        np.broadcast_to(np.asarray(vec, np.float32)[None, :], (128, len(vec))))
    common = {
        "wret": np.ascontiguousarray(np.asarray(W_ret, np.float32).astype(bf)),
        "w1": np.ascontiguousarray(np.asarray(W1, np.float32).astype(bf)),
        "w2": np.ascontiguousarray(np.asarray(W2, np.float32).astype(bf)),
        "bret": bc(b_ret), "b2": bc(b2),
        "lns1": bc(ln1_scale), "lnb1": bc(ln1_bias),
        "lns2": bc(ln2_scale), "lnb2": bc(ln2_bias),
        "b1": np.ascontiguousarray(np.asarray(b1, np.float32).reshape(KF, 128).T),
        "ema_l": Lt,
        "ident": np.eye(128, dtype=np.float32),
        "identb": np.eye(128, dtype=np.float32).astype(bf),
    }
    in_maps = []
    for core in range(N_CORES):
        b, half = divmod(core, 2)
        xs = np.empty((TCI * 128, H), np.float32)
        if half == 0:
            xs[:128] = 0.0
            xs[128:] = x[b, 0:T]
            U = np.zeros_like(Ut)
        else:
            xs[:] = x[b, T - 128:S]
            U = Ut
        m = dict(common)
        m["x"] = xs
        m["ema_u"] = Ut
        m["ema_u0"] = U
        in_maps.append(m)
    return in_maps


def gather_out(results):
    out = np.empty((B, S, H), np.float32)
    for core in range(N_CORES):
        b, half = divmod(core, 2)
        out[b, half * T:(half + 1) * T] = results[core]["out"]
    return out


class SpmdRunner:
    def __init__(self, nc, n_cores):
        install_neuronx_cc_hook()
        self.nc = nc
        self.n_cores = n_cores
        assert nc.dbg_addr is None or not nc.dbg_callbacks

        in_names, out_names, out_avals, zero_outs = [], [], [], []
        partition_name = nc.partition_id_tensor.name if nc.partition_id_tensor else None
        for alloc in nc.m.functions[0].allocations:
            if not isinstance(alloc, mybir.MemoryLocationSet):
                continue
            name = alloc.memorylocations[0].name
            if alloc.kind == "ExternalInput":
                if name != partition_name:
                    in_names.append(name)
            elif alloc.kind == "ExternalOutput":
                shape = tuple(alloc.tensor_shape)
                dtype = mybir.dt.np(alloc.dtype)
                out_names.append(name)
                out_avals.append(jax.core.ShapedArray(shape, dtype))
                zero_outs.append(np.zeros(shape, dtype))
        if nc.dbg_addr is not None:
            self.dbg_name = nc.dbg_addr.name
        else:
            self.dbg_name = None
        self.in_names = list(in_names)
        self.out_names = out_names
        self.out_avals = out_avals
        self.zero_outs = zero_outs
        self.partition_name = partition_name
        n_params = len(self.in_names)
        n_outs = len(out_names)

        all_in_names = list(self.in_names) + list(out_names)
        if partition_name is not None:
            all_in_names.append(partition_name)

        def _body(*args):
            operands = list(args)
            if partition_name is not None:
                operands.append(partition_id_tensor())
            outs = _bass_exec_p.bind(
                *operands,
                out_avals=tuple(out_avals),
                in_names=tuple(all_in_names),
                out_names=tuple(out_names),
                lowering_input_output_aliases=(),
                sim_require_finite=True,
                sim_require_nnan=True,
                nc=nc,
            )
            return tuple(outs)

        devices = jax.devices()[:n_cores]
        assert len(devices) == n_cores
        self.mesh = Mesh(np.asarray(devices), ("core",))
        in_specs = (PartitionSpec("core"),) * (n_params + n_outs)
        out_specs = (PartitionSpec("core"),) * n_outs
        self.fn = jax.jit(
            shard_map(_body, mesh=self.mesh, in_specs=in_specs,
                      out_specs=out_specs, check_rep=False),
            keep_unused=True,
        )
        self._dev_zeros = None

    def _concat(self, in_maps):
        per_core = [[np.asarray(m[name]) for name in self.in_names] for m in in_maps]
        return [np.concatenate([per_core[c][i] for c in range(self.n_cores)], axis=0)
                for i in range(len(self.in_names))]

    def put(self, in_maps):
        concat_in = self._concat(in_maps)
        dev_in = [jax.device_put(x) for x in concat_in]
        if self._dev_zeros is None:
            self._dev_zeros = [
                jax.device_put(np.zeros((self.n_cores * z.shape[0], *z.shape[1:]), z.dtype))
                for z in self.zero_outs
            ]
        return dev_in

    def run(self, dev_in):
        out = self.fn(*dev_in, *self._dev_zeros)
        jax.block_until_ready(out)
        return out

    def results(self, out_arrs):
        res = []
        for c in range(self.n_cores):
            res.append({
                name: np.asarray(out_arrs[i]).reshape(self.n_cores, *self.out_avals[i].shape)[c]
                for i, name in enumerate(self.out_names)
            })
        return res

    def time_exec(self, dev_in, n=5):
        ts = []
        for _ in range(n):
            t0 = time.perf_counter()
            self.run(dev_in)
            ts.append(time.perf_counter() - t0)
        return min(ts), ts


# ---------------------------------------------------------------------------
# Public entry point: full inputs in, full output out.
# ---------------------------------------------------------------------------

_CACHE = {}


def kernel(x, W_ret, b_ret, ln1_scale, ln1_bias, W1, b1, W2, b2,
           ln2_scale, ln2_bias):
    """CRAM block on 8 Trainium2 NeuronCores. Full [4,4096,1024] in/out."""
    if "runner" not in _CACHE:
        nc = build_nc(repeat=1)
        _CACHE["runner"] = SpmdRunner(nc, N_CORES)
    runner = _CACHE["runner"]
    in_maps = make_in_maps(x, W_ret, b_ret, ln1_scale, ln1_bias, W1, b1,
                           W2, b2, ln2_scale, ln2_bias)
    dev_in = runner.put(in_maps)
    results = runner.results(runner.run(dev_in))
    return gather_out(results).astype(np.float32)
